# revision 14
# baseline (speedup 1.0000x reference)
"""Multi-head attention (B=4, T=2048, D=1024, H=16) on 8 Trainium2 NeuronCores.

Sharding: core = (batch, head-group): b = core // 2, g = core % 2.
Each core computes heads [g*8, g*8+8) of batch b.

Per-core dataflow (PE row-charge-optimal orientation):
  - Q/K projections into transposed layout qT/kT = W_g @ x_b.T  [512, 2048]
  - V projection in natural layout [2048, 512] per head pair (+ ones column
    per head for softmax row sums)
  - scores transposed: S.T tile = K_h @ Q_h.T; head pairs (2i, 2i+1) at
    partition bases 0/64 run in distinct PE row groups
  - exp fused on ScalarE over two-bank PSUM groups, scale=1/sqrt(64)
  - PV in [queries, dims] orientation: lhsT = pt tile [keys,128q] (stationary),
    rhs = [V_h | 1] [keys, 65] -> psum [128q, 65] with row-sums in col 64.
    This charges 65 rows/matmul instead of 512 (cost ~ half of the S.T form).
  - normalize fused into the PSUM->SBUF copy (tensor_scalar_mul by 1/rowsum,
    a per-partition scalar in this orientation - no broadcast matmul needed)
  - O transposed back to [dims, tokens] via DMA-XBAR transpose (off-engine)
  - output projection y tile [tokens, douts] = O.T-tile.T @ Wo_g.T
Host: y[b] = (y_part[2b] + y_part[2b+1]) + bo', with bo' = bo + Wo @ bv
(softmax rows sum to 1, so the V bias adds exactly bv to every attention
output row; bk is dropped entirely - softmax is invariant to per-query
constant shifts q.bk).

Emission is paced: a global fill-work queue (projections, PV of the previous
chunk, transposes, trailing output projection) is drained between score/exp
pairs so the PE and ScalarE streams stay balanced end to end.

Self-contained: hardcodes all shapes; requires only concourse (bass) + numpy.
"""

import numpy as np

B, T, D = 4, 2048, 1024
H, HD = 16, 64
HG, DG = 8, 512          # heads / feature columns per core
NCORES = 8
P = 128
KD = D // P              # 8  k-tiles over model dim
MQ = DG // P             # 4  partition tiles of qT/kT (one per head pair)
TK = T // P              # 16 key tiles
TQC = 512                # query-chunk (= one fp32 PSUM bank)
NC2 = T // TQC           # 4  query chunks
NQ4 = TQC // P           # 4  query sub-tiles per chunk
VW = HD + 1              # V columns per head incl. ones column
SCALE = 0.125            # 1/sqrt(HD)

# cost-model pacing constants (warm)
_PE_NS_PER_ROW = 1.0 / 2.4
_ACT_NS_PER_SCPAIR = 2.0 * (1024 + 222) / 1.2

_CACHE: dict = {}


def _emit(tc, aps, dbg=None, reps=1):
    import concourse.bass as bass  # noqa: F401
    from concourse import mybir

    nc = tc.nc
    dt = mybir.dt
    f32, bf16 = dt.float32, dt.bfloat16
    AF = mybir.ActivationFunctionType
    xT, wq, wk, wv, wo, bq, yO = (
        aps["xT"], aps["wq"], aps["wk"], aps["wv"], aps["wo"], aps["bq"], aps["y"],
    )

    from contextlib import ExitStack

    with ExitStack() as ctx:
        const = ctx.enter_context(tc.tile_pool(name="const", bufs=1))
        persist = ctx.enter_context(tc.tile_pool(name="persist", bufs=1))
        xw = ctx.enter_context(tc.tile_pool(name="xw", bufs=1))
        ptp = ctx.enter_context(tc.tile_pool(name="ptp", bufs=2))
        osb = ctx.enter_context(tc.tile_pool(name="osb", bufs=2))
        ysb = ctx.enter_context(tc.tile_pool(name="ysb", bufs=2))
        nrm = ctx.enter_context(tc.tile_pool(name="nrm", bufs=4))
        scps = ctx.enter_context(tc.tile_pool(name="scps", bufs=2, space="PSUM"))
        qkvps = ctx.enter_context(tc.tile_pool(name="qkvps", bufs=2, space="PSUM"))
        pvps = ctx.enter_context(tc.tile_pool(name="pvps", bufs=2, space="PSUM"))

        # ---- persistent SBUF ----
        q_sb = persist.tile([P, MQ, T], bf16)
        k_sb = persist.tile([P, MQ, T], bf16)
        v_sb = persist.tile([P, TK, HG * VW], bf16)
        oT_sb = persist.tile([P, MQ, T], bf16)
        v4d = v_sb.rearrange("p t (h c) -> p t h c", h=HG)
        nc.vector.memset(v4d[:, :, :, HD : HD + 1], 1.0)

        # ---- input DMAs: chunk-granular x so the first k/q projection chunk
        # is gated by ~2 transfers, staggered across the SP and ACT queues
        x_sb = xw.tile([P, KD, T], bf16)
        wq_sb = xw.tile([P, KD, DG], bf16)
        wk_sb = xw.tile([P, KD, DG], bf16)
        wv_sb = xw.tile([P, KD, DG], bf16)
        wo_sb = const.tile([P, MQ, D], bf16)
        bq_sb = const.tile([P, MQ], f32)
        nc.sync.dma_start(out=bq_sb, in_=bq)

        def xc(n):  # x chunk n across all k-tiles
            return (x_sb[:, :, n * TQC : (n + 1) * TQC],
                    xT[:, :, n * TQC : (n + 1) * TQC])

        nc.scalar.dma_start(out=wk_sb, in_=wk)
        o, i = xc(0)
        nc.sync.dma_start(out=o, in_=i)
        nc.sync.dma_start(out=wq_sb, in_=wq)
        o, i = xc(1)
        nc.scalar.dma_start(out=o, in_=i)
        o, i = xc(2)
        nc.sync.dma_start(out=o, in_=i)
        o, i = xc(3)
        nc.scalar.dma_start(out=o, in_=i)
        nc.sync.dma_start(out=wv_sb, in_=wv)
        nc.scalar.dma_start(out=wo_sb, in_=wo)

        # ---- emission helpers ----
        def emit_qk_part(mt, n, which):
            """One T-chunk (n) of the q and/or k projection for m-tile mt."""
            sel = {
                "k": ((wk_sb, None, k_sb),),
                "q": ((wq_sb, bq_sb, q_sb),),
            }
            parts = sel["k"] + sel["q"] if which == "kq" else sel[which]
            ki_order = list(range(0, KD, 2)) + list(range(1, KD, 2))
            for w_sb, b_col, dst in parts:
                ps = qkvps.tile([P, TQC], f32, tag="qkv", name="ps_qkv")
                for idx, ki in enumerate(ki_order):
                    nc.tensor.matmul(
                        ps,
                        w_sb[:, ki, mt * P : (mt + 1) * P],
                        x_sb[:, ki, n * TQC : (n + 1) * TQC],
                        start=(idx == 0),
                        stop=(idx == KD - 1),
                    )
                d = dst[:, mt, n * TQC : (n + 1) * TQC]
                if b_col is None:
                    nc.vector.tensor_copy(d, ps)
                else:
                    nc.vector.tensor_scalar_add(d, ps, b_col[:, mt : mt + 1])

        def emit_v_tile(pr, t):
            """V projection for head pair pr, token tile t: out [128, 128]."""
            ps = qkvps.tile([P, P], f32, tag="qkv", name="ps_v")
            for ki in range(KD):
                nc.tensor.matmul(
                    ps,
                    x_sb[:, ki, t * P : (t + 1) * P],
                    wv_sb[:, ki, pr * P : (pr + 1) * P],
                    start=(ki == 0),
                    stop=(ki == KD - 1),
                )
            nc.vector.tensor_copy(
                v4d[:, t, 2 * pr : 2 * pr + 2, 0:HD],
                ps.rearrange("p (h c) -> p h c", h=2),
            )

        def emit_sc_pair(p, c, tkp, pt):
            """Packed scores + exp for heads (2p, 2p+1), key tiles 2tkp..2tkp+1."""
            tq0 = c * TQC
            scs = [
                scps.tile([P, 2, TQC], f32, tag="sc", name="sc0"),
                scps.tile([P, 2, TQC], f32, tag="sc", name="sc1"),
            ]
            for u in range(2):
                tk = 2 * tkp + u
                for i in range(2):
                    hb = i * HD
                    nc.tensor.matmul(
                        scs[i][:, u, :],
                        k_sb[hb : hb + HD, p, tk * P : (tk + 1) * P],
                        q_sb[hb : hb + HD, p, tq0 : tq0 + TQC],
                        start=True,
                        stop=True,
                    )
            for i in range(2):
                nc.scalar.activation(
                    pt[:, 2 * tkp : 2 * tkp + 2, i, :], scs[i], AF.Exp, scale=SCALE
                )

        def emit_pv_group(p, c, s, i, pt, o_c):
            """PV + normalize for head h = 2p+i, query sub-tile s of chunk c."""
            h = 2 * p + i
            pv = pvps.tile([P, VW], f32, tag="pv", name="pv")
            for tk in range(TK):
                nc.tensor.matmul(
                    pv,
                    pt[:, tk, i, s * P : (s + 1) * P],
                    v4d[:, tk, h, :],
                    start=(tk == 0),
                    stop=(tk == TK - 1),
                )
            rc = nrm.tile([P, 1], f32, name="rc")
            nc.vector.reciprocal(rc, pv[:, HD : HD + 1])
            nc.vector.tensor_scalar_mul(
                o_c[:, s, i * HD : (i + 1) * HD], pv[:, 0:HD], rc
            )
            if dbg is not None and p == 0 and c == 0 and s == 0 and i == 0:
                nc.sync.dma_start(out=dbg["pt"], in_=pt[:, :, 0, :])
                nc.sync.dma_start(out=dbg["pv"], in_=pv)
                nc.sync.dma_start(out=dbg["rc"], in_=rc)

        def emit_transpose(p, c, s, o_c):
            tt = c * NQ4 + s
            nc.sync.dma_start_transpose(
                oT_sb[:, p, tt * P : (tt + 1) * P], o_c[:, s, :]
            )

        def emit_oproj_tt(c, s):
            """Output projection for token tile tt = c*4+s: y [128, 1024]."""
            tt = c * NQ4 + s
            y_t = ysb.tile([P, D], bf16, name="y_t")
            for j in range(2):
                ys = qkvps.tile([P, TQC], f32, tag="qkv", name="ys")
                for ki in range(MQ):
                    nc.tensor.matmul(
                        ys,
                        oT_sb[:, ki, tt * P : (tt + 1) * P],
                        wo_sb[:, ki, j * TQC : (j + 1) * TQC],
                        start=(ki == 0),
                        stop=(ki == MQ - 1),
                    )
                nc.vector.tensor_copy(y_t[:, j * TQC : (j + 1) * TQC], ys)
            nc.scalar.dma_start(out=yO[:, tt, :], in_=y_t)

        # ---- paced schedule: global fill-work queue drained between sc pairs.
        # FIFO order preserves intra-queue dataflow (V before PV, transposes
        # before oproj); drain_until() forces queue items that later inline
        # emissions (scores) read from. Each iteration spreads its due work
        # evenly across the 8 score slots so neither PE nor ScalarE starves.
        state = {"filled": 0.0}
        work = []  # (due_iter, key, pe_rows, fn)

        def add(due, rows, fn, key=None):
            work.append((due, key, rows, fn))

        def run_one():
            due, key, rows, fn = work.pop(0)
            fn()
            state["filled"] += rows
            return key

        def drain_due(it):
            while work and work[0][0] <= it:
                run_one()

        def drain_until(key):
            if any(k == key for _, k, _, _ in work):
                while run_one() != key:
                    pass

        def pace(it, frac):
            target = state["due_rows"] * frac
            while work and work[0][0] <= it and state["filled"] < target:
                run_one()

        if reps > 1:
            loop_cm = tc.For_i(0, reps, 1)
            loop_cm.__enter__()

        # pre-loop: first k/q chunks so scores can start immediately
        emit_qk_part(0, 0, "k")
        emit_qk_part(0, 0, "q")

        prev = None  # (p, c, pt, o_c) pending PV of previous iteration
        it = 0
        for p in range(MQ):
            for c in range(NC2):
                # enqueue this iteration's fill work
                if prev is not None:
                    pp, pc, ppt, po_c = prev
                    for s in range(NQ4):
                        for i in range(2):
                            add(it, TK * VW,
                                (lambda pp=pp, pc=pc, s=s, i=i, ppt=ppt, po_c=po_c:
                                 emit_pv_group(pp, pc, s, i, ppt, po_c)))
                        add(it, 64,
                            (lambda pp=pp, pc=pc, s=s, po_c=po_c:
                             emit_transpose(pp, pc, s, po_c)))
                        # oproj lags its transpose by one sub-tile so the PE
                        # never waits on the DMA-transpose round trip
                        if pp == MQ - 1 and s > 0:
                            add(it, 2 * MQ * TQC + 256,
                                (lambda pc=pc, s=s: emit_oproj_tt(pc, s - 1)))
                    if pp == MQ - 1:
                        add(it, 2 * MQ * TQC + 256,
                            (lambda pc=pc: emit_oproj_tt(pc, NQ4 - 1)))
                # projections: remaining k chunks of pair 0 land in iter 0;
                # next-pair k/q spread one chunk per iteration; V for pair
                # pr+1 spread over pair pr's iterations (4 tiles each)
                if p == 0 and c == 0:
                    for n in range(1, NC2):
                        add(it, KD * TQC,
                            (lambda n=n: emit_qk_part(0, n, "k")), key=("k", 0, n))
                    for t in range(TK):
                        add(it, KD * P, (lambda t=t: emit_v_tile(0, t)))
                if p == 0 and c < NC2 - 1:
                    add(it, KD * TQC,
                        (lambda c=c: emit_qk_part(0, c + 1, "q")),
                        key=("q", 0, c + 1))
                if p < MQ - 1:
                    add(it, KD * TQC, (lambda p=p, c=c: emit_qk_part(p + 1, c, "k")),
                        key=("k", p + 1, c))
                    add(it, KD * TQC, (lambda p=p, c=c: emit_qk_part(p + 1, c, "q")),
                        key=("q", p + 1, c))
                    for t in range(NQ4 * c, NQ4 * (c + 1)):
                        add(it, KD * P,
                            (lambda p=p, t=t: emit_v_tile(p + 1, t)))

                # hard deadline: everything due before the previous iteration
                # must be emitted (pt/o_c buffer reuse, k/q availability)
                drain_due(it - 1)

                pt = ptp.tile([P, TK, 2, TQC], bf16, tag="pt", name="pt")
                o_c = osb.tile([P, NQ4, P], bf16, tag="oc", name="oc")
                drain_until(("q", p, c))
                state["filled"] = 0.0
                state["due_rows"] = sum(
                    rows for due, _, rows, _ in work if due <= it
                )
                for tkp in range(TK // 2):
                    drain_until(("k", p, tkp // 2))
                    emit_sc_pair(p, c, tkp, pt)
                    pace(it, (tkp + 1) / (TK // 2))
                prev = (p, c, pt, o_c)
                it += 1

        # tail: PV of the final iteration + trailing output projection,
        # interleaved so oproj overlaps the next sub-tile's PV + transpose
        drain_due(it - 1)
        pp, pc, ppt, po_c = prev
        for s in range(NQ4):
            for i in range(2):
                emit_pv_group(pp, pc, s, i, ppt, po_c)
            emit_transpose(pp, pc, s, po_c)
            if s > 0:
                emit_oproj_tt(pc, s - 1)
        drain_due(it)
        emit_oproj_tt(pc, NQ4 - 1)

        if reps > 1:
            loop_cm.__exit__(None, None, None)

        if dbg is not None:
            nc.sync.dma_start(out=dbg["q"], in_=q_sb)
            nc.sync.dma_start(out=dbg["k"], in_=k_sb)
            nc.sync.dma_start(out=dbg["v"], in_=v_sb)
            nc.sync.dma_start(out=dbg["o"], in_=oT_sb)


def _build(debug=False, reps=1):
    import concourse.tile as tile
    from concourse import bacc, mybir

    dt = mybir.dt
    f32, bf16 = dt.float32, dt.bfloat16

    nc = bacc.Bacc("TRN2", target_bir_lowering=False, debug=False)
    # inputs are host-preswizzled into partition-major layouts so every DMA
    # descriptor is a fat contiguous run
    aps = {
        "xT": nc.dram_tensor("xT", [P, KD, T], bf16, kind="ExternalInput").ap(),
        "wq": nc.dram_tensor("wq", [P, KD, DG], bf16, kind="ExternalInput").ap(),
        "wk": nc.dram_tensor("wk", [P, KD, DG], bf16, kind="ExternalInput").ap(),
        "wv": nc.dram_tensor("wv", [P, KD, DG], bf16, kind="ExternalInput").ap(),
        "wo": nc.dram_tensor("wo", [P, MQ, D], bf16, kind="ExternalInput").ap(),
        "bq": nc.dram_tensor("bq", [P, MQ], f32, kind="ExternalInput").ap(),
        "y": nc.dram_tensor("y", [P, TK, D], bf16, kind="ExternalOutput").ap(),
    }

    dbg = None
    if debug:
        dbg = {
            "q": nc.dram_tensor("dbg_q", [P, MQ, T], bf16, kind="ExternalOutput").ap(),
            "k": nc.dram_tensor("dbg_k", [P, MQ, T], bf16, kind="ExternalOutput").ap(),
            "v": nc.dram_tensor(
                "dbg_v", [P, TK, HG * VW], bf16, kind="ExternalOutput"
            ).ap(),
            "o": nc.dram_tensor("dbg_o", [P, MQ, T], bf16, kind="ExternalOutput").ap(),
            "pt": nc.dram_tensor(
                "dbg_pt", [P, TK, TQC], bf16, kind="ExternalOutput"
            ).ap(),
            "pv": nc.dram_tensor("dbg_pv", [P, VW], f32, kind="ExternalOutput").ap(),
            "rc": nc.dram_tensor("dbg_rc", [P, 1], f32, kind="ExternalOutput").ap(),
        }

    with tile.TileContext(nc) as tc:
        _emit(tc, aps, dbg, reps=reps)
    nc.compile()
    return nc


def _get_nc():
    if "nc" not in _CACHE:
        _CACHE["nc"] = _build()
    return _CACHE["nc"]


def _shard_inputs(x, Wq, bq, Wk, bk, Wv, bv, Wo, bo):
    import ml_dtypes

    bf16 = ml_dtypes.bfloat16
    f32 = np.float32

    def c(a, dtype):
        return np.ascontiguousarray(a).astype(dtype)

    def kp(a, kt):  # [kt*P, F] -> [P, kt, F] partition-major swizzle
        return a.reshape(kt, P, a.shape[-1]).transpose(1, 0, 2)

    in_maps = []
    for core in range(NCORES):
        b, g = core // 2, core % 2
        hs = g * DG
        in_maps.append(
            {
                "xT": c(kp(x[b].T, KD), bf16),
                "wq": c(kp(Wq[hs : hs + DG, :].T, KD), bf16),
                "wk": c(kp(Wk[hs : hs + DG, :].T, KD), bf16),
                "wv": c(kp(Wv[hs : hs + DG, :].T, KD), bf16),
                "wo": c(kp(Wo[:, hs : hs + DG].T, MQ), bf16),
                "bq": c(bq[hs : hs + DG].reshape(MQ, P).T, f32),
            }
        )
    return in_maps


def _run(inputs, trace=False):
    from concourse import bass_utils

    nc = _get_nc()
    np_in = {k: np.asarray(v) for k, v in inputs.items()}
    in_maps = _shard_inputs(**np_in)
    res = bass_utils.run_bass_kernel_spmd(
        nc, in_maps, core_ids=list(range(NCORES)), trace=trace
    )
    # bk drops out of softmax exactly; bv folds into the output bias:
    # softmax rows sum to 1 so the V bias adds bv to every attention output.
    bo_eff = (
        np_in["bo"].astype(np.float32)
        + np_in["Wo"].astype(np.float32) @ np_in["bv"].astype(np.float32)
    )
    y = np.empty((B, T, D), dtype=np.float32)
    for b in range(B):
        acc = res.results[2 * b]["y"].astype(np.float32) + res.results[2 * b + 1][
            "y"
        ].astype(np.float32)  # [P, TK, D]
        y[b] = acc.transpose(1, 0, 2).reshape(T, D) + bo_eff
    return y, res


def kernel(**inputs):
    y, _ = _run(inputs)
    return y


# revision 15
# speedup vs baseline: 1.0172x; 1.0172x over previous
"""Multi-head attention (B=4, T=2048, D=1024, H=16) on 8 Trainium2 NeuronCores.

Sharding: core = (batch, head-group): b = core // 2, g = core % 2.
Each core computes heads [g*8, g*8+8) of batch b.

Per-core dataflow (PE row-charge-optimal orientation):
  - Q/K projections into transposed layout qT/kT = W_g @ x_b.T  [512, 2048]
  - V projection in natural layout [2048, 512] per head pair (+ ones column
    per head for softmax row sums)
  - scores transposed: S.T tile = K_h @ Q_h.T; head pairs (2i, 2i+1) at
    partition bases 0/64 run in distinct PE row groups
  - exp fused on ScalarE over two-bank PSUM groups, scale=1/sqrt(64)
  - PV in [queries, dims] orientation: lhsT = pt tile [keys,128q] (stationary),
    rhs = [V_h | 1] [keys, 65] -> psum [128q, 65] with row-sums in col 64.
    This charges 65 rows/matmul instead of 512 (cost ~ half of the S.T form).
  - normalize fused into the PSUM->SBUF copy (tensor_scalar_mul by 1/rowsum,
    a per-partition scalar in this orientation - no broadcast matmul needed)
  - O transposed back to [dims, tokens] via DMA-XBAR transpose (off-engine)
  - output projection y tile [tokens, douts] = O.T-tile.T @ Wo_g.T
Host: y[b] = (y_part[2b] + y_part[2b+1]) + bo', with bo' = bo + Wo @ bv
(softmax rows sum to 1, so the V bias adds exactly bv to every attention
output row; bk is dropped entirely - softmax is invariant to per-query
constant shifts q.bk).

Emission is paced: a global fill-work queue (projections, PV of the previous
chunk, transposes, trailing output projection) is drained between score/exp
pairs so the PE and ScalarE streams stay balanced end to end.

Self-contained: hardcodes all shapes; requires only concourse (bass) + numpy.
"""

import numpy as np

B, T, D = 4, 2048, 1024
H, HD = 16, 64
HG, DG = 8, 512          # heads / feature columns per core
NCORES = 8
P = 128
KD = D // P              # 8  k-tiles over model dim
MQ = DG // P             # 4  partition tiles of qT/kT (one per head pair)
TK = T // P              # 16 key tiles
TQC = 512                # query-chunk (= one fp32 PSUM bank)
NC2 = T // TQC           # 4  query chunks
NQ4 = TQC // P           # 4  query sub-tiles per chunk
VW = HD + 1              # V columns per head incl. ones column
SCALE = 0.125            # 1/sqrt(HD)

# cost-model pacing constants (warm)
_PE_NS_PER_ROW = 1.0 / 2.4
_ACT_NS_PER_SCPAIR = 2.0 * (1024 + 222) / 1.2

_CACHE: dict = {}


def _emit(tc, aps, dbg=None, reps=1):
    import concourse.bass as bass  # noqa: F401
    from concourse import mybir

    nc = tc.nc
    dt = mybir.dt
    f32, bf16 = dt.float32, dt.bfloat16
    AF = mybir.ActivationFunctionType
    xT, wq, wk, wv, wo, bq, yO = (
        aps["xT"], aps["wq"], aps["wk"], aps["wv"], aps["wo"], aps["bq"], aps["y"],
    )

    from contextlib import ExitStack

    with ExitStack() as ctx:
        const = ctx.enter_context(tc.tile_pool(name="const", bufs=1))
        persist = ctx.enter_context(tc.tile_pool(name="persist", bufs=1))
        xw = ctx.enter_context(tc.tile_pool(name="xw", bufs=1))
        ptp = ctx.enter_context(tc.tile_pool(name="ptp", bufs=2))
        osb = ctx.enter_context(tc.tile_pool(name="osb", bufs=2))
        ysb = ctx.enter_context(tc.tile_pool(name="ysb", bufs=2))
        nrm = ctx.enter_context(tc.tile_pool(name="nrm", bufs=4))
        scps = ctx.enter_context(tc.tile_pool(name="scps", bufs=2, space="PSUM"))
        qkvps = ctx.enter_context(tc.tile_pool(name="qkvps", bufs=2, space="PSUM"))
        pvps = ctx.enter_context(tc.tile_pool(name="pvps", bufs=2, space="PSUM"))

        # ---- persistent SBUF ----
        q_sb = persist.tile([P, MQ, T], bf16)
        k_sb = persist.tile([P, MQ, T], bf16)
        v_sb = persist.tile([P, TK, HG * VW], bf16)
        oT_sb = persist.tile([P, MQ, T], bf16)
        v4d = v_sb.rearrange("p t (h c) -> p t h c", h=HG)
        nc.vector.memset(v4d[:, :, :, HD : HD + 1], 1.0)

        # ---- input DMAs: chunk-granular x so the first k/q projection chunk
        # is gated by ~2 transfers, staggered across the SP and ACT queues
        x_sb = xw.tile([P, KD, T], bf16)
        wq_sb = xw.tile([P, KD, DG], bf16)
        wk_sb = xw.tile([P, KD, DG], bf16)
        wv_sb = xw.tile([P, KD, DG], bf16)
        wo_sb = const.tile([P, MQ, D], bf16)
        bq_sb = const.tile([P, MQ], f32)
        nc.sync.dma_start(out=bq_sb, in_=bq)

        def xc(n):  # x chunk n across all k-tiles
            return (x_sb[:, :, n * TQC : (n + 1) * TQC],
                    xT[:, :, n * TQC : (n + 1) * TQC])

        nc.scalar.dma_start(out=wk_sb, in_=wk)
        o, i = xc(0)
        nc.sync.dma_start(out=o, in_=i)
        nc.sync.dma_start(out=wq_sb, in_=wq)
        o, i = xc(1)
        nc.scalar.dma_start(out=o, in_=i)
        o, i = xc(2)
        nc.sync.dma_start(out=o, in_=i)
        o, i = xc(3)
        nc.scalar.dma_start(out=o, in_=i)
        nc.sync.dma_start(out=wv_sb, in_=wv)
        nc.scalar.dma_start(out=wo_sb, in_=wo)

        # ---- emission helpers ----
        def emit_qk_part(mt, n, which):
            """One T-chunk (n) of the q and/or k projection for m-tile mt."""
            sel = {
                "k": ((wk_sb, None, k_sb),),
                "q": ((wq_sb, bq_sb, q_sb),),
            }
            parts = sel["k"] + sel["q"] if which == "kq" else sel[which]
            ki_order = list(range(0, KD, 2)) + list(range(1, KD, 2))
            for w_sb, b_col, dst in parts:
                ps = qkvps.tile([P, TQC], f32, tag="qkv", name="ps_qkv")
                for idx, ki in enumerate(ki_order):
                    nc.tensor.matmul(
                        ps,
                        w_sb[:, ki, mt * P : (mt + 1) * P],
                        x_sb[:, ki, n * TQC : (n + 1) * TQC],
                        start=(idx == 0),
                        stop=(idx == KD - 1),
                    )
                d = dst[:, mt, n * TQC : (n + 1) * TQC]
                if b_col is None:
                    nc.vector.tensor_copy(d, ps)
                else:
                    nc.vector.tensor_scalar_add(d, ps, b_col[:, mt : mt + 1])

        def emit_v_tile(pr, t):
            """V projection for head pair pr, token tile t: out [128, 128]."""
            ps = qkvps.tile([P, P], f32, tag="qkv", name="ps_v")
            for ki in range(KD):
                nc.tensor.matmul(
                    ps,
                    x_sb[:, ki, t * P : (t + 1) * P],
                    wv_sb[:, ki, pr * P : (pr + 1) * P],
                    start=(ki == 0),
                    stop=(ki == KD - 1),
                )
            nc.vector.tensor_copy(
                v4d[:, t, 2 * pr : 2 * pr + 2, 0:HD],
                ps.rearrange("p (h c) -> p h c", h=2),
            )

        def emit_sc_pair(p, c, tkp, pt):
            """Packed scores + exp for heads (2p, 2p+1), key tiles 2tkp..2tkp+1."""
            tq0 = c * TQC
            scs = [
                scps.tile([P, 2, TQC], f32, tag="sc", name="sc0"),
                scps.tile([P, 2, TQC], f32, tag="sc", name="sc1"),
            ]
            for u in range(2):
                tk = 2 * tkp + u
                for i in range(2):
                    hb = i * HD
                    nc.tensor.matmul(
                        scs[i][:, u, :],
                        k_sb[hb : hb + HD, p, tk * P : (tk + 1) * P],
                        q_sb[hb : hb + HD, p, tq0 : tq0 + TQC],
                        start=True,
                        stop=True,
                    )
            for i in range(2):
                nc.scalar.activation(
                    pt[:, 2 * tkp : 2 * tkp + 2, i, :], scs[i], AF.Exp, scale=SCALE
                )

        def emit_pv_group(p, c, s, i, pt, o_c):
            """PV + normalize for head h = 2p+i, query sub-tile s of chunk c."""
            h = 2 * p + i
            pv = pvps.tile([P, VW], f32, tag="pv", name="pv")
            for tk in range(TK):
                nc.tensor.matmul(
                    pv,
                    pt[:, tk, i, s * P : (s + 1) * P],
                    v4d[:, tk, h, :],
                    start=(tk == 0),
                    stop=(tk == TK - 1),
                )
            rc = nrm.tile([P, 1], f32, name="rc")
            nc.vector.reciprocal(rc, pv[:, HD : HD + 1])
            nc.vector.tensor_scalar_mul(
                o_c[:, s, i * HD : (i + 1) * HD], pv[:, 0:HD], rc
            )
            if dbg is not None and p == 0 and c == 0 and s == 0 and i == 0:
                nc.sync.dma_start(out=dbg["pt"], in_=pt[:, :, 0, :])
                nc.sync.dma_start(out=dbg["pv"], in_=pv)
                nc.sync.dma_start(out=dbg["rc"], in_=rc)

        def emit_transpose(p, c, s, o_c):
            tt = c * NQ4 + s
            nc.sync.dma_start_transpose(
                oT_sb[:, p, tt * P : (tt + 1) * P], o_c[:, s, :]
            )

        def emit_oproj_tt(c, s):
            """Output projection for token tile tt = c*4+s: y [128, 1024]."""
            tt = c * NQ4 + s
            y_t = ysb.tile([P, D], bf16, name="y_t")
            for j in range(2):
                ys = qkvps.tile([P, TQC], f32, tag="qkv", name="ys")
                for ki in range(MQ):
                    nc.tensor.matmul(
                        ys,
                        oT_sb[:, ki, tt * P : (tt + 1) * P],
                        wo_sb[:, ki, j * TQC : (j + 1) * TQC],
                        start=(ki == 0),
                        stop=(ki == MQ - 1),
                    )
                nc.vector.tensor_copy(y_t[:, j * TQC : (j + 1) * TQC], ys)
            # y DMA on the idle gpsimd (SWDGE) queue: its sem-wait would
            # head-of-line-block the exp stream on the ACT queue's sequencer
            nc.gpsimd.dma_start(out=yO[:, tt, :], in_=y_t)

        # ---- paced schedule: global fill-work queue drained between sc pairs.
        # FIFO order preserves intra-queue dataflow (V before PV, transposes
        # before oproj); drain_until() forces queue items that later inline
        # emissions (scores) read from. Each iteration spreads its due work
        # evenly across the 8 score slots so neither PE nor ScalarE starves.
        state = {"filled": 0.0}
        work = []  # (due_iter, key, pe_rows, fn)

        def add(due, rows, fn, key=None):
            work.append((due, key, rows, fn))

        def run_one():
            due, key, rows, fn = work.pop(0)
            fn()
            state["filled"] += rows
            return key

        def drain_due(it):
            while work and work[0][0] <= it:
                run_one()

        def drain_until(key):
            if any(k == key for _, k, _, _ in work):
                while run_one() != key:
                    pass

        def pace(it, frac):
            target = state["due_rows"] * frac
            while work and work[0][0] <= it and state["filled"] < target:
                run_one()

        if reps > 1:
            loop_cm = tc.For_i(0, reps, 1)
            loop_cm.__enter__()

        # pre-loop: first k/q chunks so scores can start immediately
        emit_qk_part(0, 0, "k")
        emit_qk_part(0, 0, "q")

        prev = None  # (p, c, pt, o_c) pending PV of previous iteration
        it = 0
        for p in range(MQ):
            for c in range(NC2):
                # enqueue this iteration's fill work
                if prev is not None:
                    pp, pc, ppt, po_c = prev
                    for s in range(NQ4):
                        for i in range(2):
                            add(it, TK * VW,
                                (lambda pp=pp, pc=pc, s=s, i=i, ppt=ppt, po_c=po_c:
                                 emit_pv_group(pp, pc, s, i, ppt, po_c)))
                        add(it, 64,
                            (lambda pp=pp, pc=pc, s=s, po_c=po_c:
                             emit_transpose(pp, pc, s, po_c)))
                        # oproj lags its transpose by one sub-tile so the PE
                        # never waits on the DMA-transpose round trip
                        if pp == MQ - 1 and s > 0:
                            add(it, 2 * MQ * TQC + 256,
                                (lambda pc=pc, s=s: emit_oproj_tt(pc, s - 1)))
                    if pp == MQ - 1:
                        add(it, 2 * MQ * TQC + 256,
                            (lambda pc=pc: emit_oproj_tt(pc, NQ4 - 1)))
                # projections: remaining k chunks of pair 0 land in iter 0;
                # next-pair k/q spread one chunk per iteration; V for pair
                # pr+1 spread over pair pr's iterations (4 tiles each)
                if p == 0 and c == 0:
                    for n in range(1, NC2):
                        add(it, KD * TQC,
                            (lambda n=n: emit_qk_part(0, n, "k")), key=("k", 0, n))
                    for t in range(TK):
                        add(it, KD * P, (lambda t=t: emit_v_tile(0, t)))
                if p == 0 and c < NC2 - 1:
                    add(it, KD * TQC,
                        (lambda c=c: emit_qk_part(0, c + 1, "q")),
                        key=("q", 0, c + 1))
                if p < MQ - 1:
                    add(it, KD * TQC, (lambda p=p, c=c: emit_qk_part(p + 1, c, "k")),
                        key=("k", p + 1, c))
                    add(it, KD * TQC, (lambda p=p, c=c: emit_qk_part(p + 1, c, "q")),
                        key=("q", p + 1, c))
                    for t in range(NQ4 * c, NQ4 * (c + 1)):
                        add(it, KD * P,
                            (lambda p=p, t=t: emit_v_tile(p + 1, t)))

                # hard deadline: everything due before the previous iteration
                # must be emitted (pt/o_c buffer reuse, k/q availability)
                drain_due(it - 1)

                pt = ptp.tile([P, TK, 2, TQC], bf16, tag="pt", name="pt")
                o_c = osb.tile([P, NQ4, P], bf16, tag="oc", name="oc")
                drain_until(("q", p, c))
                state["filled"] = 0.0
                state["due_rows"] = sum(
                    rows for due, _, rows, _ in work if due <= it
                )
                for tkp in range(TK // 2):
                    drain_until(("k", p, tkp // 2))
                    emit_sc_pair(p, c, tkp, pt)
                    pace(it, (tkp + 1) / (TK // 2))
                prev = (p, c, pt, o_c)
                it += 1

        # tail: PV of the final iteration + trailing output projection,
        # interleaved so oproj overlaps the next sub-tile's PV + transpose
        drain_due(it - 1)
        pp, pc, ppt, po_c = prev
        for s in range(NQ4):
            for i in range(2):
                emit_pv_group(pp, pc, s, i, ppt, po_c)
            emit_transpose(pp, pc, s, po_c)
            if s > 0:
                emit_oproj_tt(pc, s - 1)
        drain_due(it)
        emit_oproj_tt(pc, NQ4 - 1)

        if reps > 1:
            loop_cm.__exit__(None, None, None)

        if dbg is not None:
            nc.sync.dma_start(out=dbg["q"], in_=q_sb)
            nc.sync.dma_start(out=dbg["k"], in_=k_sb)
            nc.sync.dma_start(out=dbg["v"], in_=v_sb)
            nc.sync.dma_start(out=dbg["o"], in_=oT_sb)


def _build(debug=False, reps=1):
    import concourse.tile as tile
    from concourse import bacc, mybir

    dt = mybir.dt
    f32, bf16 = dt.float32, dt.bfloat16

    nc = bacc.Bacc("TRN2", target_bir_lowering=False, debug=False)
    # inputs are host-preswizzled into partition-major layouts so every DMA
    # descriptor is a fat contiguous run
    aps = {
        "xT": nc.dram_tensor("xT", [P, KD, T], bf16, kind="ExternalInput").ap(),
        "wq": nc.dram_tensor("wq", [P, KD, DG], bf16, kind="ExternalInput").ap(),
        "wk": nc.dram_tensor("wk", [P, KD, DG], bf16, kind="ExternalInput").ap(),
        "wv": nc.dram_tensor("wv", [P, KD, DG], bf16, kind="ExternalInput").ap(),
        "wo": nc.dram_tensor("wo", [P, MQ, D], bf16, kind="ExternalInput").ap(),
        "bq": nc.dram_tensor("bq", [P, MQ], f32, kind="ExternalInput").ap(),
        "y": nc.dram_tensor("y", [P, TK, D], bf16, kind="ExternalOutput").ap(),
    }

    dbg = None
    if debug:
        dbg = {
            "q": nc.dram_tensor("dbg_q", [P, MQ, T], bf16, kind="ExternalOutput").ap(),
            "k": nc.dram_tensor("dbg_k", [P, MQ, T], bf16, kind="ExternalOutput").ap(),
            "v": nc.dram_tensor(
                "dbg_v", [P, TK, HG * VW], bf16, kind="ExternalOutput"
            ).ap(),
            "o": nc.dram_tensor("dbg_o", [P, MQ, T], bf16, kind="ExternalOutput").ap(),
            "pt": nc.dram_tensor(
                "dbg_pt", [P, TK, TQC], bf16, kind="ExternalOutput"
            ).ap(),
            "pv": nc.dram_tensor("dbg_pv", [P, VW], f32, kind="ExternalOutput").ap(),
            "rc": nc.dram_tensor("dbg_rc", [P, 1], f32, kind="ExternalOutput").ap(),
        }

    with tile.TileContext(nc) as tc:
        _emit(tc, aps, dbg, reps=reps)
    nc.compile()
    return nc


def _get_nc():
    if "nc" not in _CACHE:
        _CACHE["nc"] = _build()
    return _CACHE["nc"]


def _shard_inputs(x, Wq, bq, Wk, bk, Wv, bv, Wo, bo):
    import ml_dtypes

    bf16 = ml_dtypes.bfloat16
    f32 = np.float32

    def c(a, dtype):
        return np.ascontiguousarray(a).astype(dtype)

    def kp(a, kt):  # [kt*P, F] -> [P, kt, F] partition-major swizzle
        return a.reshape(kt, P, a.shape[-1]).transpose(1, 0, 2)

    in_maps = []
    for core in range(NCORES):
        b, g = core // 2, core % 2
        hs = g * DG
        in_maps.append(
            {
                "xT": c(kp(x[b].T, KD), bf16),
                "wq": c(kp(Wq[hs : hs + DG, :].T, KD), bf16),
                "wk": c(kp(Wk[hs : hs + DG, :].T, KD), bf16),
                "wv": c(kp(Wv[hs : hs + DG, :].T, KD), bf16),
                "wo": c(kp(Wo[:, hs : hs + DG].T, MQ), bf16),
                "bq": c(bq[hs : hs + DG].reshape(MQ, P).T, f32),
            }
        )
    return in_maps


def _run(inputs, trace=False):
    from concourse import bass_utils

    nc = _get_nc()
    np_in = {k: np.asarray(v) for k, v in inputs.items()}
    in_maps = _shard_inputs(**np_in)
    res = bass_utils.run_bass_kernel_spmd(
        nc, in_maps, core_ids=list(range(NCORES)), trace=trace
    )
    # bk drops out of softmax exactly; bv folds into the output bias:
    # softmax rows sum to 1 so the V bias adds bv to every attention output.
    bo_eff = (
        np_in["bo"].astype(np.float32)
        + np_in["Wo"].astype(np.float32) @ np_in["bv"].astype(np.float32)
    )
    y = np.empty((B, T, D), dtype=np.float32)
    for b in range(B):
        acc = res.results[2 * b]["y"].astype(np.float32) + res.results[2 * b + 1][
            "y"
        ].astype(np.float32)  # [P, TK, D]
        y[b] = acc.transpose(1, 0, 2).reshape(T, D) + bo_eff
    return y, res


def kernel(**inputs):
    y, _ = _run(inputs)
    return y
